# revision 7
# baseline (speedup 1.0000x reference)
"""Multi-head causal attention (Whisper-style) on 8 trn2 NeuronCores, v2.

Sharding: batch x head-quad.  Cores 0-3 take batch 0, cores 4-7 batch 1;
core c handles heads 4*(c%4) .. 4*(c%4)+3 (E=256 feature columns) of its
batch.  Each core gets xT for ONE batch (bf16), its column slice of
Wq/Wk/Wv (+bias slices), its row slice of Wo, and emits a [1024, S]
partial yT (bf16).  The host sums 4 partials per batch and adds bo.

vs v1: halves x/y DMA and the out-proj PSUM-drain volume, and packs the
two heads of each head-pair into concurrent PE row-tiles for the scores
matmuls (contraction=64 -> tile_position (0,0) / (64,0), adjacent issue).

PSUM (8 banks): psS 2x[128,2,512] scores (4) + psO 2x[65,1,512] per-head
o^T accumulators (2) + psP 2x[128,1,512] proj/vtrans/outproj (2).

Attention runs per (head-pair hp, q-tile j): for each contributing
k-chunk: packed scores pair -> ONE exp over the flat [o:1024] span (the
h1 left-of-diagonal garbage is exp'd but never read) -> per-head o^T
accumulate with sub-span diagonal handling.  Softmax denominator via
ones-columns in vn ([v0|1|v1|1] layout).  Projections / v-transposes /
out-proj units are injected into the attention passes to fill PE idle.
"""

import os
import sys
from contextlib import ExitStack

import numpy as np

for _p in ("/root/.axon_site/_ro/trn_rl_repo", "/opt/trn_rl_repo"):
    if os.path.isdir(_p) and _p not in sys.path:
        sys.path.append(_p)

import concourse.bass as bass
import concourse.mybir as mybir
import concourse.tile as tile
from concourse import bacc, bass_utils

F32 = mybir.dt.float32
F32R = mybir.dt.float32r
BF16 = mybir.dt.bfloat16
AF = mybir.ActivationFunctionType
ALU = mybir.AluOpType

N_STATE = 1024
N_HEAD = 16
HD = 64
N_CORES = 8
B_GLOBAL = 2
HP = 2                      # head-pairs per core
E = 4 * HD                  # 256 feature columns per core (4 heads)
Q_TILE = 512
K_CHUNK = 128
G_TILE = 1024               # stage-A token group (2 q-tiles)
N_D = N_STATE // 128        # 8 contraction chunks for the projections
SCALE = float(HD) ** -0.25
NEG_THRESH = -50.0
SPLIT32 = False             # 4-way scores row-tiling: fails at runtime on this stack


def classify_blocks(maskT):
    """Value-driven classification of (k_chunk, q_tile) mask blocks."""
    S = maskT.shape[0]
    cls = {}
    for ki in range(S // K_CHUNK):
        for j in range(S // Q_TILE):
            blk = maskT[ki * K_CHUNK:(ki + 1) * K_CHUNK,
                        j * Q_TILE:(j + 1) * Q_TILE]
            if np.all(blk < NEG_THRESH):
                cls[(ki, j)] = "skip"
            elif np.all(blk == 0.0):
                cls[(ki, j)] = "clean"
            else:
                cls[(ki, j)] = "partial"
                k_idx = ki * K_CHUNK + np.arange(K_CHUNK)[:, None]
                q_idx = j * Q_TILE + np.arange(Q_TILE)[None, :]
                ok = (blk > NEG_THRESH) == (k_idx <= q_idx)
                assert ok.all(), f"partial block {(ki, j)} not causal"
    return cls


def build_kernel(S, cls, repeats=1):
    n_k = S // K_CHUNK
    n_q = S // Q_TILE
    n_g = S // G_TILE

    nc = bacc.Bacc("TRN2", target_bir_lowering=False, debug=False,
                   num_devices=N_CORES)

    xT_d = nc.dram_tensor("xT", [N_STATE, S], BF16, kind="ExternalInput")
    tri_d = nc.dram_tensor("tri", [K_CHUNK, K_CHUNK], BF16, kind="ExternalInput")
    wq_d = nc.dram_tensor("wq", [N_STATE, E], BF16, kind="ExternalInput")
    wk_d = nc.dram_tensor("wk", [N_STATE, E], BF16, kind="ExternalInput")
    wv_d = nc.dram_tensor("wv", [N_STATE, E], BF16, kind="ExternalInput")
    wo_d = nc.dram_tensor("wo", [E, N_STATE], BF16, kind="ExternalInput")
    bq_d = nc.dram_tensor("bq", [E], F32, kind="ExternalInput")
    ident_d = nc.dram_tensor("ident", [128, 128], BF16, kind="ExternalInput")
    bv_d = nc.dram_tensor("bv", [E], F32, kind="ExternalInput")
    yT_d = nc.dram_tensor("yT", [N_STATE, S], BF16, kind="ExternalOutput")

    # last contributing k-chunk per q-tile
    last_ki = {j: max(ki for ki in range(n_k) if cls[(ki, j)] != "skip")
               for j in range(n_q)}

    with tile.TileContext(nc) as tc, ExitStack() as ctx:
        const = ctx.enter_context(tc.tile_pool(name="const", bufs=1))
        xpool = ctx.enter_context(tc.tile_pool(name="xpool", bufs=2))
        vstage = ctx.enter_context(tc.tile_pool(name="vstage", bufs=2))
        wexp = ctx.enter_context(tc.tile_pool(name="wexp", bufs=6))
        otsb = ctx.enter_context(tc.tile_pool(name="otsb", bufs=4))
        nrm = ctx.enter_context(tc.tile_pool(name="nrm", bufs=6))
        yspool = ctx.enter_context(tc.tile_pool(name="yspool", bufs=6))
        psS = ctx.enter_context(tc.tile_pool(name="psS", bufs=2, space="PSUM"))
        psO = ctx.enter_context(tc.tile_pool(name="psO", bufs=2, space="PSUM"))
        psP = ctx.enter_context(tc.tile_pool(name="psP", bufs=2, space="PSUM"))

        # ---- resident constants / weights (emitted in first-use order) ----
        wq_sb = const.tile([128, N_D, E], BF16, tag="wq_sb")
        wk_sb = const.tile([128, N_D, E], BF16, tag="wk_sb")
        wv_sb = const.tile([128, N_D, E], BF16, tag="wv_sb")
        # chunk-split, interleaved with the x chunks inside u_dma(g0) so
        # the first projection matmul is gated on ~640KB, not 2.5MB
        wq_src = wq_d[:].rearrange("(c p) e -> p c e", p=128)

        def wq_part(c2):
            nc.sync.dma_start(wq_sb[:, c2:c2 + 2, :], wq_src[:, c2:c2 + 2, :])
        bq_sb = const.tile([128, HP], F32, tag="bq_sb")
        bv_sb = const.tile([128, HP], F32, tag="bv_sb")
        tri = const.tile([K_CHUNK, K_CHUNK], BF16, tag="tri")
        ident = const.tile([128, 128], BF16, tag="ident")

        qT = const.tile([128, HP, S], F32R, tag="qT")
        kT = const.tile([128, HP, S], F32R, tag="kT")
        onT = const.tile([128, HP, S], BF16, tag="onT")
        # v + ones columns per head-pair: [h0 64 | 1 | h1 64 | 1]
        vn = const.tile([128, HP, n_k, 2 * (HD + 1)], BF16, tag="vn")

        wo_sb = None

        def load_rest_consts():
            nonlocal wo_sb
            # chunk-split so the first k-projection isn't gated on the
            # full wk transfer
            wk_src = wk_d[:].rearrange("(c p) e -> p c e", p=128)
            for c2 in range(0, N_D, 4):
                nc.sync.dma_start(wk_sb[:, c2:c2 + 4, :],
                                    wk_src[:, c2:c2 + 4, :])
            nc.sync.dma_start(wv_sb[:],
                                wv_d[:].rearrange("(c p) e -> p c e", p=128))
            nc.sync.dma_start(bq_sb[:], bq_d[:].rearrange("(h p) -> p h", p=128))
            nc.sync.dma_start(bv_sb[:], bv_d[:].rearrange("(h p) -> p h", p=128))
            nc.sync.dma_start(tri[:], tri_d[:])
            nc.sync.dma_start(ident[:], ident_d[:])
            wo_sb = const.tile([128, HP, N_STATE], BF16, tag="wo_sb")
            nc.sync.dma_start(wo_sb[:],
                                wo_d[:].rearrange("(h p) m -> p h m", p=128))
            nc.gpsimd.memset(vn[:], 1.0)  # ones columns persist (Pool)

        first_head = [True]

        # ---------------- stage A: projections for one token group ----------
        def a_units(g):
            """Injectable atoms for token group g: 1 DMA + 12 proj halves
            + 2 vtrans units."""
            ts0 = g * G_TILE
            xt = [None]
            vs_box = {}

            def u_dma():
                xt[0] = xpool.tile([128, N_D, G_TILE], BF16, tag="xt",
                                   name=f"xt{g}")
                src = xT_d[:, ts0:ts0 + G_TILE].rearrange(
                    "(c p) t -> p c t", p=128)
                if g == 0 and first_head[0]:
                    # x chunks alternate SP-HWDGE / Pool-SWDGE: two DMA
                    # paths run the prologue transfer in parallel
                    for i2, c2 in enumerate(range(0, N_D, 2)):
                        wq_part(c2)
                        eng = nc.sync if i2 % 2 == 0 else nc.gpsimd
                        eng.dma_start(xt[0][:, c2:c2 + 2, :],
                                      src[:, c2:c2 + 2, :])
                elif g == 0:
                    for i2, c2 in enumerate(range(0, N_D, 2)):
                        eng = nc.sync if i2 % 2 == 0 else nc.gpsimd
                        eng.dma_start(xt[0][:, c2:c2 + 2, :],
                                      src[:, c2:c2 + 2, :])
                else:
                    nc.sync.dma_start(xt[0][:], src)

            def u_proj(which, hp, a):
                ts = slice(ts0 + a * Q_TILE, ts0 + (a + 1) * Q_TILE)
                w_sb = {"q": wq_sb, "k": wk_sb, "v": wv_sb}[which]
                es = slice(hp * 128, (hp + 1) * 128)
                ps = psP.tile([128, 1, Q_TILE], F32, tag="pp")
                for c in range(N_D):
                    nc.tensor.matmul(
                        ps[:, 0, :], w_sb[:, c, es],
                        xt[0][:, c, a * Q_TILE:(a + 1) * Q_TILE],
                        start=(c == 0), stop=(c == N_D - 1))
                if which == "q":
                    nc.vector.tensor_scalar(qT[:, hp, ts], ps[:, 0, :],
                                            bq_sb[:, hp:hp + 1], SCALE,
                                            ALU.add, ALU.mult)
                elif which == "k":
                    nc.vector.tensor_scalar(kT[:, hp, ts], ps[:, 0, :],
                                            SCALE, None, ALU.mult)
                else:
                    if (hp, g) not in vs_box:
                        vs_box[(hp, g)] = vstage.tile(
                            [128, G_TILE], BF16, tag="vs", name=f"vs{hp}_{g}")
                    nc.vector.tensor_scalar(
                        vs_box[(hp, g)][:, a * Q_TILE:(a + 1) * Q_TILE],
                        ps[:, 0, :], bv_sb[:, hp:hp + 1], None, ALU.add)

            def u_vt(hp, a):
                # PE-transpose 4 token chunks (one q-tile) of vs into a
                # bitcast psP view, then 2 strided bf16 copies into vn
                nt = Q_TILE // 128
                tp = psP.tile([128, 1, Q_TILE], F32, tag="pp")
                tpb = tp[:].bitcast(BF16).rearrange(
                    "p o (c t) -> p (o c) t", t=128)
                vs = vs_box[(hp, g)]
                for t in range(nt):
                    ta = a * Q_TILE + t * 128
                    nc.tensor.matmul(tpb[:, t, :], vs[:, ta:ta + 128],
                                     ident[:], is_transpose=True)
                cs = slice(g * (G_TILE // 128) + a * nt,
                           g * (G_TILE // 128) + (a + 1) * nt)
                nc.vector.tensor_copy(vn[:, hp, cs, 0:HD], tpb[:, 0:nt, 0:HD])
                nc.vector.tensor_copy(vn[:, hp, cs, HD + 1:2 * HD + 1],
                                      tpb[:, 0:nt, HD:2 * HD])

            units = [u_dma]
            for hp in range(HP):
                for a in range(2):
                    for which in ("q", "k", "v"):
                        units.append(
                            (lambda w_, h_, a_: lambda: u_proj(w_, h_, a_))(
                                which, hp, a))
                    units.append((lambda h_, a_: lambda: u_vt(h_, a_))(hp, a))
            return units

        # ---------------- stage B: one (head-pair, q-tile) ------------------
        def b_pass(hp, j, inject=None, defer_drain=False, tail=False,
                   act_copy=False):
            inject = list(inject or ())
            n_it = sum(1 for ki in range(n_k) if cls[(ki, j)] != "skip")
            it_left = n_it
            qs0 = j * Q_TILE
            ot = [psO.tile([HD + 1, 1, Q_TILE], F32, tag="ot",
                           name=f"ot{hp}_{j}_{h}") for h in range(2)]

            def emit_ot(ki, wt2, o):
                sub = o > 0
                for h in range(2):
                    vslice = slice(h * (HD + 1), (h + 1) * (HD + 1))
                    nc.tensor.matmul(ot[h][:, 0, o:], vn[:, hp, ki, vslice],
                                     wt2[:, h, o:],
                                     start=(ki == 0), stop=(ki == last_ki[j]),
                                     skip_group_check=sub)

            pending = None  # 1-deep SW pipeline: o^T lags scores by a unit
            for ki in range(n_k):
                part = cls[(ki, j)]
                if part == "skip":
                    continue
                quota = -(-len(inject) // it_left) if inject else 0
                it_left -= 1
                sc = psS.tile([128, 2, Q_TILE], F32, tag="sc")
                scf = sc[:].rearrange("p a q -> p (a q)")
                ks = slice(ki * K_CHUNK, (ki + 1) * K_CHUNK)
                o = ki * K_CHUNK - qs0 if part == "partial" else 0
                lo_s = o if Q_TILE - o >= 256 else 0
                qs = slice(qs0 + lo_s, qs0 + Q_TILE)
                # 4-way row packing: the 64-deep contraction of each head
                # splits into two 32-row PE tiles that ACCUMULATE into the
                # same PSUM region; tiles at (0,0)/(32,0) (h0) run
                # concurrently with (64,0)/(96,0) (h1).  The second tile of
                # each pair trails the first by the ~4ns dispatch stagger,
                # so its read-modify-write lands strictly after the reset.
                if SPLIT32:
                    for h in range(2):
                        for i2, (r0, st) in enumerate(((0, True), (32, False))):
                            rs = slice(h * HD + r0, h * HD + r0 + 32)
                            nc.tensor.matmul(
                                sc[:, h, lo_s:], kT[rs, hp, ks],
                                qT[rs, hp, qs], start=st, stop=not st,
                                tile_position=(h * HD + r0, 0),
                                skip_group_check=True)
                else:
                    nc.tensor.matmul(sc[:, 0, lo_s:], kT[0:HD, hp, ks],
                                     qT[0:HD, hp, qs], start=True, stop=True)
                    nc.tensor.matmul(sc[:, 1, lo_s:], kT[HD:128, hp, ks],
                                     qT[HD:128, hp, qs], start=True, stop=True)
                wt2 = wexp.tile([K_CHUNK, 2, Q_TILE], BF16, tag="wexp")
                wt2f = wt2[:].rearrange("p a q -> p (a q)")
                if pending is not None:
                    emit_ot(*pending)
                # ONE exp over the flat [o:1024] span; h1's [512:512+o]
                # garbage lands in wt2 but is never streamed by o^T
                nc.scalar.activation(wt2f[:, o:], scf[:, o:], AF.Exp)
                if part == "partial":
                    for h in range(2):
                        nc.vector.tensor_tensor(wt2[:, h, o:o + K_CHUNK],
                                                wt2[:, h, o:o + K_CHUNK],
                                                tri[:], ALU.mult)
                # injects go AFTER the scores/exp emission: the injected
                # matmuls then fill the PE while ACT runs this ki's exp
                for _ in range(quota):
                    if inject:
                        inject.pop(0)()
                pending = (ki, wt2, o)
            if pending is not None:
                emit_ot(*pending)
            while inject:
                inject.pop(0)()
            # drain: PSUM -> SBUF bf16, then normalize (deferred closures)
            osb = otsb.tile([HD + 1, 2, Q_TILE], BF16, tag="osb")

            def d_copy():
                nc.vector.tensor_copy(osb[:, 0, :], ot[0][:, 0, :])
                if tail or act_copy:  # run h1's copy on ACT in parallel
                    nc.scalar.activation(osb[:, 1, :], ot[1][:, 0, :], AF.Copy)
                else:
                    nc.vector.tensor_copy(osb[:, 1, :], ot[1][:, 0, :])

            def d_norm():
                qs = slice(qs0, qs0 + Q_TILE)
                rd = nrm.tile([1, 2, Q_TILE], BF16, tag="rd")
                with nc.allow_low_precision(reason="softmax denom, 2e-2 gate"):
                    nc.vector.reciprocal(rd[:], osb[HD:HD + 1, :, :])
                if tail:
                    # replace the 2.3us broadcast-DMA round trip with a PE
                    # ones-row matmul into a (tail-free) psS bank
                    bcp = psS.tile([128, 2, Q_TILE], F32, tag="sc")
                    for h in range(2):
                        nc.tensor.matmul(bcp[:, h, :], tri[0:1, :],
                                         rd[0:1, h, :], start=True, stop=True)
                    nc.vector.tensor_tensor(onT[0:HD, hp, qs], osb[0:HD, 0, :],
                                            bcp[0:HD, 0, :], ALU.mult)
                    nc.vector.tensor_tensor(onT[HD:128, hp, qs],
                                            osb[0:HD, 1, :],
                                            bcp[0:HD, 1, :], ALU.mult)
                    return
                bc = nrm.tile([HD, 2, Q_TILE], BF16, tag="bc")
                for h in range(2):
                    rdap = rd[0:1, h, :]
                    rd_rep = bass.AP(rdap.tensor, rdap.offset,
                                     [list(rdap.ap[0]), [0, HD],
                                      list(rdap.ap[1])])
                    # latency-sensitive 1KB broadcast: keep it off the
                    # SP queue (busy with bulk x transfers)
                    nc.scalar.dma_start(bc[:, h, :], rd_rep)
                nc.vector.tensor_tensor(onT[0:HD, hp, qs], osb[0:HD, 0, :],
                                        bc[:, 0, :], ALU.mult)
                nc.vector.tensor_tensor(onT[HD:128, hp, qs], osb[0:HD, 1, :],
                                        bc[:, 1, :], ALU.mult)

            drain = [d_copy, d_norm]
            if defer_drain:
                return drain
            for d in drain:
                d()
            return []

        # ---------------- stage C: out-proj for (m-chunk, q-tile) -----------
        def c_unit(m, j, copy_eng, dma_eng=None):
            ms = slice(m * 128, (m + 1) * 128)
            qs = slice(j * Q_TILE, (j + 1) * Q_TILE)
            yp = psP.tile([128, 1, Q_TILE], F32, tag="pp")
            for hp in range(HP):
                nc.tensor.matmul(yp[:, 0, :], wo_sb[:, hp, ms],
                                 onT[:, hp, qs],
                                 start=(hp == 0), stop=(hp == HP - 1))
            ys = yspool.tile([128, Q_TILE], BF16, tag="ys")
            # GPSIMD cannot read PSUM: drain on DVE or ACT; y DMA issues
            # from the otherwise-idle Pool SWDGE queue (HWDGE at the tail).
            if copy_eng == "dve":
                nc.vector.tensor_copy(ys[:], yp[:, 0, :])
            else:
                nc.scalar.activation(ys[:], yp[:, 0, :], AF.Copy)
            (dma_eng or nc.gpsimd).dma_start(yT_d[ms, qs], ys[:])

        def c_units(j, engs=("dve", "act"), dma_engs=(None,)):
            return [(lambda m_, e_, de_: (lambda: c_unit(m_, j, e_, de_)))(
                        m, engs[m % len(engs)], dma_engs[m % len(dma_engs)])
                    for m in range(N_STATE // 128)]

        def emit_head(first):
            """Minimal inline prologue: x DMAs + hp0's first-half projs.
            Everything else is handed back as per-pass inject lists, so
            the first attention pass starts ~4 units in."""
            a0 = a_units(0)
            a0[0]()  # x(g0) DMA, wq-interleaved on the first head
            first_head[0] = False
            if first:
                load_rest_consts()
            a1 = a_units(1)
            a1[0]()  # x(g1) DMA
            for u in a0[1:5]:  # hp0 a0: q,k,v,vt -- gates pass (0,0)
                u()
            return [a0[9:13], a0[5:9], a0[13:17],
                    a1[1:5], a1[9:13], a1[5:9], a1[13:17]]

        def run_body(au, last):
            d = b_pass(0, 0, inject=au[0], defer_drain=True)
            d = b_pass(1, 0, inject=d + au[1], defer_drain=True)
            d = b_pass(0, 1, inject=d + au[2], defer_drain=True)
            d = b_pass(1, 1, inject=d + au[3], defer_drain=True)
            cu0 = c_units(0, engs=("dve",))
            d = b_pass(0, 2, inject=d + au[4] + cu0, defer_drain=True)
            cu1 = c_units(1, engs=("dve",))  # mid-kernel drains stay off ACT (exp pacer)
            d = b_pass(1, 2, inject=d + au[5] + cu1, defer_drain=True)
            # late c2 units drain on ACT (exp-free by then) so their psP
            # slots free promptly, and their y DMAs avoid the backed-up
            # Pool SWDGE ring
            cu2 = c_units(2, engs=("dve", "dve", "dve", "dve",
                                   "act", "act", "act", "act"),
                          dma_engs=(None, None, None, None,
                                    nc.sync, nc.scalar, nc.sync, nc.scalar))
            d = b_pass(0, 3, inject=d + au[6], defer_drain=True)
            d = b_pass(1, 3, inject=d + cu2[:2], defer_drain=True, tail=last)
            nxt = None
            if not last:
                # prefetch next repeat's g0 head ahead of the final drains
                nxt = emit_head(False)
            # reserved c2 units fill the PE while the final drain chain
            # (d_copy -> recip -> PE bcast -> norm) resolves
            d[0]()
            for u in cu2[2:7]:
                u()
            d[1]()
            cu2[7]()
            for u in c_units(3, engs=("dve", "act"),
                             dma_engs=(nc.sync, nc.scalar)):
                u()
            return nxt

        a1u = emit_head(True)
        for _rep in range(repeats):
            a1u = run_body(a1u, _rep == repeats - 1)

    nc.finalize()
    return nc


def shard_inputs(x, Wq, bq, Wk, Wv, bv, Wo):
    """Per-core input dicts: batch x head-quad slicing."""
    bf16 = mybir.dt.np(BF16)
    xT = np.ascontiguousarray(x.transpose(0, 2, 1)).astype(bf16)
    S = x.shape[1]
    i = np.arange(K_CHUNK)
    tri = (i[:, None] <= i[None, :]).astype(bf16)
    in_maps = []
    for c in range(N_CORES):
        b, qd = divmod(c, N_CORES // B_GLOBAL)
        cs = slice(qd * E, (qd + 1) * E)
        in_maps.append({
            "xT": xT[b],
            "tri": tri,
            "wq": np.ascontiguousarray(Wq[:, cs]).astype(bf16),
            "wk": np.ascontiguousarray(Wk[:, cs]).astype(bf16),
            "wv": np.ascontiguousarray(Wv[:, cs]).astype(bf16),
            "wo": np.ascontiguousarray(Wo[cs, :]).astype(bf16),
            "bq": np.ascontiguousarray(bq[cs]).astype(np.float32),
            "bv": np.ascontiguousarray(bv[cs]).astype(np.float32),
            "ident": np.eye(128).astype(bf16),
        })
    return in_maps


_NC_CACHE = {}


def _get_nc(S, cls_key, cls, repeats=1):
    key = (S, cls_key, repeats)
    if key not in _NC_CACHE:
        _NC_CACHE[key] = build_kernel(S, cls, repeats=repeats)
    return _NC_CACHE[key]


def _gather(results, B, S, bo):
    acc = np.zeros((B, N_STATE, S), dtype=np.float64)
    for c, r in enumerate(results):
        acc[c // (N_CORES // B_GLOBAL)] += r["yT"].astype(np.float64)
    return (acc.transpose(0, 2, 1).astype(np.float32)
            + np.asarray(bo, np.float32))


def run(x, mask, Wq, bq, Wk, Wv, bv, Wo, bo, trace=False):
    B, S, D = x.shape
    maskT = np.ascontiguousarray(np.asarray(mask).T).astype(np.float32)
    cls = classify_blocks(maskT)
    cls_key = tuple(sorted((k, v) for k, v in cls.items()))
    nc = _get_nc(S, hash(cls_key), cls)
    in_maps = shard_inputs(np.asarray(x, np.float32),
                           np.asarray(Wq, np.float32), np.asarray(bq, np.float32),
                           np.asarray(Wk, np.float32), np.asarray(Wv, np.float32),
                           np.asarray(bv, np.float32), np.asarray(Wo, np.float32))
    res = bass_utils.run_bass_kernel_spmd(
        nc, in_maps, core_ids=list(range(N_CORES)), trace=trace)
    return _gather(res.results, B, S, bo), res


def kernel(x, mask, Wq, bq, Wk, Wv, bv, Wo, bo):
    y, _ = run(x, mask, Wq, bq, Wk, Wv, bv, Wo, bo, trace=False)
    return y


def time_run(x, mask, Wq, bq, Wk, Wv, bv, Wo, bo, iters=20, repeats=1):
    """Device-resident timing of the SPMD program (see v1 docstring)."""
    import time as _time
    import jax
    from jax.experimental.shard_map import shard_map
    from jax.sharding import Mesh, NamedSharding, PartitionSpec
    from concourse import bass2jax
    from concourse.bass2jax import _bass_exec_p, install_neuronx_cc_hook

    install_neuronx_cc_hook()
    B, S, D = x.shape
    maskT = np.ascontiguousarray(np.asarray(mask).T).astype(np.float32)
    cls = classify_blocks(maskT)
    cls_key = tuple(sorted((k, v) for k, v in cls.items()))
    nc = _get_nc(S, hash(cls_key), cls, repeats=repeats)
    in_maps = shard_inputs(np.asarray(x, np.float32),
                           np.asarray(Wq, np.float32), np.asarray(bq, np.float32),
                           np.asarray(Wk, np.float32), np.asarray(Wv, np.float32),
                           np.asarray(bv, np.float32), np.asarray(Wo, np.float32))

    in_names, out_names, out_avals, zero_outs = [], [], [], []
    partition_name = (nc.partition_id_tensor.name
                      if nc.partition_id_tensor else None)
    for alloc in nc.m.functions[0].allocations:
        if not isinstance(alloc, mybir.MemoryLocationSet):
            continue
        name = alloc.memorylocations[0].name
        if alloc.kind == "ExternalInput":
            if name != partition_name:
                in_names.append(name)
        elif alloc.kind == "ExternalOutput":
            out_names.append(name)
            shape = tuple(alloc.tensor_shape)
            dtype = mybir.dt.np(alloc.dtype)
            out_avals.append((shape, dtype))
            zero_outs.append(np.zeros(shape, dtype))
    n_params = len(in_names)
    n_outs = len(out_names)
    all_in_names = list(in_names) + list(out_names)
    if partition_name is not None:
        all_in_names.append(partition_name)

    def _body(*args):
        operands = list(args)
        if partition_name is not None:
            operands.append(bass2jax.partition_id_tensor())
        outs = _bass_exec_p.bind(
            *operands,
            out_avals=tuple(
                jax.core.ShapedArray(s, d) for s, d in out_avals),
            in_names=tuple(all_in_names),
            out_names=tuple(out_names),
            lowering_input_output_aliases=(),
            sim_require_finite=True,
            sim_require_nnan=True,
            nc=nc,
        )
        return tuple(outs)

    devices = jax.devices()[:N_CORES]
    mesh = Mesh(np.asarray(devices), ("core",))
    spec = PartitionSpec("core")
    donate = tuple(range(n_params, n_params + n_outs))
    sharded = jax.jit(
        shard_map(_body, mesh=mesh, in_specs=(spec,) * (n_params + n_outs),
                  out_specs=(spec,) * n_outs, check_rep=False),
        donate_argnums=donate, keep_unused=True)

    sh = NamedSharding(mesh, spec)
    dev_in = [
        jax.device_put(
            np.concatenate([np.asarray(in_maps[c][nm]) for c in range(N_CORES)],
                           axis=0), sh)
        for nm in in_names
    ]
    out = sharded(*dev_in, *[
        jax.device_put(np.zeros((N_CORES * z.shape[0], *z.shape[1:]), z.dtype),
                       sh) for z in zero_outs])
    jax.block_until_ready(out)  # warmup + compile
    t0 = _time.perf_counter()
    for _ in range(iters):
        out = sharded(*dev_in, *out)
    jax.block_until_ready(out)
    dt = (_time.perf_counter() - t0) / iters

    yT_all = np.asarray(out[out_names.index("yT")]).reshape(
        N_CORES, N_STATE, S)
    y = _gather([{"yT": yT_all[c]} for c in range(N_CORES)], B, S, bo)
    return y, dt
